# revision 3
# baseline (speedup 1.0000x reference)
"""Trainium2 Bass kernel for nn_CrossAttention (self-attention, B=2, N=4096,
QD=512, 8 heads x 64 dim).

Sharding: 16 (batch, head) pairs across 8 cores -> core c handles batch c//4
and heads {2*(c%4), 2*(c%4)+1}.  Projection weights are column-sliced (Wq/Wk/Wv)
and row-sliced (Wo) per core; each core emits a partial [4096, 512] output that
the host sums per batch (row-parallel Wo => all-reduce done on host at gather).

Device kernel (per core, 2 heads packed on 128 partitions). ScalarE exp is the
bottleneck engine (~1.15us per j-tile ACTIVATE over [128,1024]); everything
else is scheduled to keep ACT 100% busy:
  - per j-tile: row-tiled QK^T pair (K=64 heads at row groups 0/64) -> S^T
    [128j, 512i] per head in one 2-bank PSUM group; one exp ACTIVATE over
    [128, 1024] (scale fused, no max subtraction; |S| <~ 1.5); AV matmuls lag
    LAG j-groups.  V' has a ones column (65th) so softmax denominators fall
    out of the AV matmul (row 64).
  - slice epilogue is 1-slice-deep and front-loaded: av->SBUF copies at j=0,
    fast-approx reciprocals (custom DVE op, ~5x faster than iterative divide)
    at j=2, GPSIMD partition broadcasts at j=4, normalize muls at j=6, next
    slice's q projection at j=8 (so its DVE copy clears the FIFO mid-slice and
    the next slice's first QK never waits), Wo matmuls at j=16..22.  This
    keeps the DVE FIFO shallow so the cross-slice chain never blocks the PE
    queue at slice boundaries (the previous version lost ~7us/slice there and
    HAM-rethrottled the PE to 1.2GHz).
  - DMA order: wk + x slice 0 first so the k projection starts ~10us earlier.
"""

import sys

sys.path.insert(0, "/opt/trn_rl_repo")

import numpy as np
import ml_dtypes

import concourse.bass as bass
import concourse.mybir as mybir
from concourse import bacc
from concourse.tile import TileContext
from concourse.bass_utils import run_bass_kernel_spmd
from concourse.masks import make_identity

B, N, QD = 2, 4096, 512
HEADS, DIM_HEAD = 8, 64
INNER = HEADS * DIM_HEAD
SCALE = DIM_HEAD**-0.5

NCORES = 8
HPC = 2  # heads per core
D2 = HPC * DIM_HEAD  # 128
KT = 4  # k tiles of 128 over QD=512
ISL = 512  # i slice
NI = N // ISL  # 8
JTL = 128  # j tile
NJ = N // JTL  # 32
LAG = 4  # AV matmuls trail QK/exp by this many j-groups

USE_FAST_RECIP = False

F32 = mybir.dt.float32
BF16 = mybir.dt.bfloat16
BFNP = ml_dtypes.bfloat16
EXP = mybir.ActivationFunctionType.Exp


def build_program():
    nc = bacc.Bacc("TRN2", target_bir_lowering=False, debug=False,
                   num_devices=NCORES)

    xT = nc.dram_tensor("xT", [QD, N], BF16, kind="ExternalInput").ap()
    wq = nc.dram_tensor("wq", [QD, D2], BF16, kind="ExternalInput").ap()
    wk = nc.dram_tensor("wk", [QD, D2], BF16, kind="ExternalInput").ap()
    wv = nc.dram_tensor("wv", [QD, D2], BF16, kind="ExternalInput").ap()
    wo = nc.dram_tensor("wo", [D2, QD], BF16, kind="ExternalInput").ap()
    out = nc.dram_tensor("out", [N, QD], F32, kind="ExternalOutput").ap()

    with TileContext(nc) as tc:
        with tc.tile_pool(name="persist", bufs=1) as pp, \
             tc.tile_pool(name="st_ps", bufs=2, space="PSUM") as st_ps, \
             tc.tile_pool(name="av_ps", bufs=1, space="PSUM") as av_ps, \
             tc.tile_pool(name="aux_ps", bufs=1, space="PSUM") as aux_ps, \
             tc.tile_pool(name="p_sb", bufs=8) as p_sb, \
             tc.tile_pool(name="n_sb", bufs=2) as n_sb:
            x_sb = pp.tile([128, KT, N], BF16)
            wq_sb = pp.tile([128, KT, D2], BF16)
            wk_sb = pp.tile([128, KT, D2], BF16)
            wv_sb = pp.tile([128, KT, D2], BF16)
            wo_sb = pp.tile([128, QD], BF16)
            ident = pp.tile([128, 128], BF16)
            qT = pp.tile([128, N], BF16)
            kT = pp.tile([128, N], BF16)
            vT = pp.tile([128, N], BF16)
            v0p = pp.tile([128, NJ, DIM_HEAD + 1], BF16)
            v1p = pp.tile([128, NJ, DIM_HEAD + 1], BF16)

            xTr = xT.rearrange("(k p) n -> p k n", p=128)
            # wk + x slice 0 gate the first projection: issue them first.
            nc.sync.dma_start(out=wk_sb[:], in_=wk.rearrange("(k p) m -> p k m", p=128))
            nc.sync.dma_start(out=x_sb[:, :, 0:ISL], in_=xTr[:, :, 0:ISL])
            nc.sync.dma_start(out=wq_sb[:], in_=wq.rearrange("(k p) m -> p k m", p=128))
            nc.sync.dma_start(out=wv_sb[:], in_=wv.rearrange("(k p) m -> p k m", p=128))
            for s in range(1, NI):
                ssl = slice(s * ISL, (s + 1) * ISL)
                nc.sync.dma_start(out=x_sb[:, :, ssl], in_=xTr[:, :, ssl])
            nc.sync.dma_start(out=wo_sb[:], in_=wo[:])
            make_identity(nc, ident[:])
            nc.gpsimd.memset(v0p[:, :, DIM_HEAD], 1.0)
            nc.gpsimd.memset(v1p[:, :, DIM_HEAD], 1.0)

            def proj(w_sb, dst, s):
                """dst[:, s*ISL:(s+1)*ISL] = (W^T @ x^T) slice, via aux psum."""
                ssl = slice(s * ISL, (s + 1) * ISL)
                ps = aux_ps.tile([128, ISL], F32, tag="ps")
                for k in range(KT):
                    nc.tensor.matmul(ps[:], w_sb[:, k, :], x_sb[:, k, ssl],
                                     start=(k == 0), stop=(k == KT - 1))
                nc.vector.tensor_copy(out=dst[:, ssl], in_=ps[:])

            def transp(j):
                """V'[j] tiles from vT via PE transpose (both heads)."""
                tp = aux_ps.tile([128, 128], BF16, tag="aux")
                nc.tensor.transpose(tp[:], vT[:, j * JTL:(j + 1) * JTL], ident[:])
                nc.vector.tensor_copy(out=v0p[:, j, 0:DIM_HEAD], in_=tp[:, 0:DIM_HEAD])
                nc.vector.tensor_copy(out=v1p[:, j, 0:DIM_HEAD], in_=tp[:, DIM_HEAD:D2])

            states = {}

            def emit_epilogue_step(i_prev, step):
                """Deferred epilogue for slice i_prev (runs during slice
                i_prev+1): PSUM release, normalize, Wo + store."""
                e = states[i_prev]
                if step == 0:
                    e["av_sb0"] = n_sb.tile([DIM_HEAD + 1, ISL], F32, tag="av_sb0", name="av_sb0")
                    e["av_sb1"] = n_sb.tile([DIM_HEAD + 1, ISL], F32, tag="av_sb1", name="av_sb1")
                    nc.vector.tensor_copy(out=e["av_sb0"][:], in_=e["av0"][:])
                    nc.vector.tensor_copy(out=e["av_sb1"][:], in_=e["av1"][:])
                elif step == 1:
                    e["r0"] = n_sb.tile([1, ISL], F32, tag="r0", name="r0")
                    e["r1"] = n_sb.tile([1, ISL], F32, tag="r1", name="r1")
                    if USE_FAST_RECIP:
                        nc.vector.reciprocal_approx_fast(
                            out=e["r0"][:], in_=e["av_sb0"][DIM_HEAD:DIM_HEAD + 1, :])
                        nc.vector.reciprocal_approx_fast(
                            out=e["r1"][:], in_=e["av_sb1"][DIM_HEAD:DIM_HEAD + 1, :])
                    else:
                        nc.vector.reciprocal(e["r0"][:], e["av_sb0"][DIM_HEAD:DIM_HEAD + 1, :])
                        nc.vector.reciprocal(e["r1"][:], e["av_sb1"][DIM_HEAD:DIM_HEAD + 1, :])
                elif step == 2:
                    e["r0b"] = n_sb.tile([64, ISL], F32, tag="r0b", name="r0b")
                    e["r1b"] = n_sb.tile([64, ISL], F32, tag="r1b", name="r1b")
                    nc.gpsimd.partition_broadcast(e["r0b"][:], e["r0"][:])
                    nc.gpsimd.partition_broadcast(e["r1b"][:], e["r1"][:])
                elif step == 3:
                    e["lh"] = n_sb.tile([128, ISL], BF16, tag="lh", name="lh")
                    nc.vector.tensor_mul(out=e["lh"][0:64, :],
                                         in0=e["av_sb0"][0:DIM_HEAD, :], in1=e["r0b"][:])
                    nc.vector.tensor_mul(out=e["lh"][64:128, :],
                                         in0=e["av_sb1"][0:DIM_HEAD, :], in1=e["r1b"][:])
                else:  # steps 4..7: one Wo matmul + store each
                    s = step - 4
                    wop = aux_ps.tile([128, QD], F32, tag="aux")
                    nc.tensor.matmul(wop[:], e["lh"][:, s * 128:(s + 1) * 128],
                                     wo_sb[:], start=True, stop=True)
                    wos = n_sb.tile([128, QD], F32, tag="wos")
                    nc.vector.tensor_copy(out=wos[:], in_=wop[:])
                    nc.sync.dma_start(
                        out=out[i_prev * ISL + s * 128:i_prev * ISL + (s + 1) * 128, :],
                        in_=wos[:])

            # warm up slice 0 of each projection before the attention loop
            proj(wk_sb, kT, 0)
            proj(wq_sb, qT, 0)
            proj(wv_sb, vT, 0)
            for j in range(4):
                transp(j)

            # epilogue emission slots within the following slice's j-loop
            EPI = {0: 0, 2: 1, 4: 2, 6: 3, 16: 4, 18: 5, 20: 6, 22: 7}

            for i in range(NI):
                isl = slice(i * ISL, (i + 1) * ISL)
                av0 = av_ps.tile([DIM_HEAD + 1, ISL], F32, tag="av0")
                av1 = av_ps.tile([DIM_HEAD + 1, ISL], F32, tag="av1")
                pts = {}
                for j in range(NJ + LAG):
                    if j < NJ:
                        jsl = slice(j * JTL, (j + 1) * JTL)
                        st = st_ps.tile([128, 2 * ISL], F32, tag="st")
                        nc.tensor.matmul(st[:, 0:ISL], kT[0:64, jsl], qT[0:64, isl],
                                         start=True, stop=True)
                        nc.tensor.matmul(st[:, ISL:2 * ISL], kT[64:128, jsl],
                                         qT[64:128, isl], start=True, stop=True)
                        pt = p_sb.tile([128, 2 * ISL], BF16, tag="pt")
                        nc.scalar.activation(pt[:], st[:], EXP, scale=SCALE)
                        pts[j] = pt
                    if j >= LAG:
                        ja = j - LAG
                        pt = pts.pop(ja)
                        nc.tensor.matmul(av0[:], v0p[:, ja, :], pt[:, 0:ISL],
                                         start=(ja == 0), stop=(ja == NJ - 1))
                        nc.tensor.matmul(av1[:], v1p[:, ja, :], pt[:, ISL:2 * ISL],
                                         start=(ja == 0), stop=(ja == NJ - 1))
                    # prologue interleaves (i == 0): stream k/v/V' production
                    if i == 0 and j < NJ and j % 4 in (1, 2, 3):
                        s = j // 4 + 1
                        if s < NI:
                            if j % 4 == 1:
                                proj(wk_sb, kT, s)
                            elif j % 4 == 2:
                                proj(wv_sb, vT, s)
                            else:
                                for jj in range(4 * s, 4 * s + 4):
                                    transp(jj)
                    # deferred epilogue of slice i-1 (1-slice-deep pipeline)
                    if i > 0 and j in EPI:
                        emit_epilogue_step(i - 1, EPI[j])
                    # next slice's q projection (its DVE copy lands mid-slice,
                    # well before the next slice's first QK needs it)
                    if j == 8 and i + 1 < NI:
                        proj(wq_sb, qT, i + 1)
                states[i] = {"av0": av0, "av1": av1}

            # drain: full epilogue for the last slice
            for step in range(8):
                emit_epilogue_step(NI - 1, step)

    nc.compile()
    return nc


_NC = None


def _get_program():
    global _NC
    if _NC is None:
        _NC = build_program()
    return _NC


def kernel(x, Wq, Wk, Wv, Wo, bo):
    x = np.asarray(x, dtype=np.float32)
    Wq = np.asarray(Wq, dtype=np.float32)
    Wk = np.asarray(Wk, dtype=np.float32)
    Wv = np.asarray(Wv, dtype=np.float32)
    Wo = np.asarray(Wo, dtype=np.float32)
    bo = np.asarray(bo, dtype=np.float32)

    nc = _get_program()

    in_maps = []
    for c in range(NCORES):
        b, m = divmod(c, NCORES // B)
        cs = slice(m * D2, (m + 1) * D2)
        in_maps.append({
            "xT": np.ascontiguousarray(x[b].T).astype(BFNP),
            "wq": np.ascontiguousarray(Wq[:, cs]).astype(BFNP),
            "wk": np.ascontiguousarray(Wk[:, cs]).astype(BFNP),
            "wv": np.ascontiguousarray(Wv[:, cs]).astype(BFNP),
            "wo": np.ascontiguousarray(Wo[cs, :]).astype(BFNP),
        })

    res = run_bass_kernel_spmd(nc, in_maps, core_ids=list(range(NCORES)))

    out = np.zeros((B, N, QD), dtype=np.float32)
    for c in range(NCORES):
        b = c // (NCORES // B)
        out[b] += res.results[c]["out"]
    out += bo[None, None, :]
    return out


# revision 5
# speedup vs baseline: 1.1244x; 1.1244x over previous
"""Trainium2 Bass kernel for nn_CrossAttention (self-attention, B=2, N=4096,
QD=512, 8 heads x 64 dim).

Sharding: 16 (batch, head) pairs across 8 cores -> core c handles batch c//4
and heads {2*(c%4), 2*(c%4)+1}.  Projection weights are column-sliced (Wq/Wk/Wv)
and row-sliced (Wo) per core; each core emits a partial [4096, 512] output that
the host sums per batch (row-parallel Wo => all-reduce done on host at gather).

Device kernel (per core, 2 heads packed on 128 partitions). ScalarE exp is the
bottleneck engine (~1.15us per j-tile ACTIVATE over [128,1024]); everything
else is scheduled to keep ACT 100% busy:
  - per j-tile: row-tiled QK^T pair (K=64 heads at row groups 0/64) -> S^T
    [128j, 512i] per head in one 2-bank PSUM group; one exp ACTIVATE over
    [128, 1024] (scale fused, no max subtraction; |S| <~ 1.5); AV matmuls lag
    LAG j-groups.  V' has a ones column (65th) so softmax denominators fall
    out of the AV matmul (row 64).
  - slice epilogue is 1-slice-deep and front-loaded: av->SBUF copies at j=0,
    fast-approx reciprocals (custom DVE op, ~5x faster than iterative divide)
    at j=2, GPSIMD partition broadcasts at j=4, normalize muls at j=6, next
    slice's q projection at j=8 (so its DVE copy clears the FIFO mid-slice and
    the next slice's first QK never waits), Wo matmuls at j=16..22.  This
    keeps the DVE FIFO shallow so the cross-slice chain never blocks the PE
    queue at slice boundaries (the previous version lost ~7us/slice there and
    HAM-rethrottled the PE to 1.2GHz).
  - DMA order: wk + x slice 0 first so the k projection starts ~10us earlier.
"""

import sys

sys.path.insert(0, "/opt/trn_rl_repo")

import numpy as np
import ml_dtypes

import concourse.bass as bass
import concourse.mybir as mybir
from concourse import bacc
from concourse.tile import TileContext
from concourse.bass_utils import run_bass_kernel_spmd
from concourse.masks import make_identity

B, N, QD = 2, 4096, 512
HEADS, DIM_HEAD = 8, 64
INNER = HEADS * DIM_HEAD
SCALE = DIM_HEAD**-0.5

NCORES = 8
HPC = 2  # heads per core
D2 = HPC * DIM_HEAD  # 128
KT = 4  # k tiles of 128 over QD=512
ISL = 512  # i slice
NI = N // ISL  # 8
JTL = 128  # j tile
NJ = N // JTL  # 32
LAG = 4  # AV matmuls trail QK/exp by this many j-groups

USE_FAST_RECIP = False

F32 = mybir.dt.float32
BF16 = mybir.dt.bfloat16
BFNP = ml_dtypes.bfloat16
EXP = mybir.ActivationFunctionType.Exp


def build_program():
    nc = bacc.Bacc("TRN2", target_bir_lowering=False, debug=False,
                   num_devices=NCORES)

    xT = nc.dram_tensor("xT", [QD, N], BF16, kind="ExternalInput").ap()
    wq = nc.dram_tensor("wq", [QD, D2], BF16, kind="ExternalInput").ap()
    wk = nc.dram_tensor("wk", [QD, D2], BF16, kind="ExternalInput").ap()
    wv = nc.dram_tensor("wv", [QD, D2], BF16, kind="ExternalInput").ap()
    wo = nc.dram_tensor("wo", [D2, QD], BF16, kind="ExternalInput").ap()
    out = nc.dram_tensor("out", [N, QD], F32, kind="ExternalOutput").ap()

    with TileContext(nc) as tc:
        with tc.tile_pool(name="persist", bufs=1) as pp, \
             tc.tile_pool(name="st_ps", bufs=2, space="PSUM") as st_ps, \
             tc.tile_pool(name="av_ps", bufs=1, space="PSUM") as av_ps, \
             tc.tile_pool(name="aux_ps", bufs=1, space="PSUM") as aux_ps, \
             tc.tile_pool(name="p_sb", bufs=8) as p_sb, \
             tc.tile_pool(name="n_sb", bufs=2) as n_sb:
            x_sb = pp.tile([128, KT, N], BF16)
            wq_sb = pp.tile([128, KT, D2], BF16)
            wk_sb = pp.tile([128, KT, D2], BF16)
            wv_sb = pp.tile([128, KT, D2], BF16)
            wo_sb = pp.tile([128, QD], BF16)
            ident = pp.tile([128, 128], BF16)
            qT = pp.tile([128, N], BF16)
            kT = pp.tile([128, N], BF16)
            vT = pp.tile([128, N], BF16)
            v0p = pp.tile([128, NJ, DIM_HEAD + 1], BF16)
            v1p = pp.tile([128, NJ, DIM_HEAD + 1], BF16)

            xTr = xT.rearrange("(k p) n -> p k n", p=128)
            # wk + x slice 0 gate the first projection: issue them first.
            nc.sync.dma_start(out=wk_sb[:], in_=wk.rearrange("(k p) m -> p k m", p=128))
            nc.sync.dma_start(out=x_sb[:, :, 0:ISL], in_=xTr[:, :, 0:ISL])
            nc.sync.dma_start(out=wq_sb[:], in_=wq.rearrange("(k p) m -> p k m", p=128))
            nc.sync.dma_start(out=wv_sb[:], in_=wv.rearrange("(k p) m -> p k m", p=128))
            for s in range(1, NI):
                ssl = slice(s * ISL, (s + 1) * ISL)
                nc.sync.dma_start(out=x_sb[:, :, ssl], in_=xTr[:, :, ssl])
            nc.sync.dma_start(out=wo_sb[:], in_=wo[:])
            make_identity(nc, ident[:])
            nc.gpsimd.memset(v0p[:, :, DIM_HEAD], 1.0)
            nc.gpsimd.memset(v1p[:, :, DIM_HEAD], 1.0)

            def proj(w_sb, dst, s):
                """dst[:, s*ISL:(s+1)*ISL] = (W^T @ x^T) slice, via aux psum."""
                ssl = slice(s * ISL, (s + 1) * ISL)
                ps = aux_ps.tile([128, ISL], F32, tag="ps")
                for k in range(KT):
                    nc.tensor.matmul(ps[:], w_sb[:, k, :], x_sb[:, k, ssl],
                                     start=(k == 0), stop=(k == KT - 1))
                nc.vector.tensor_copy(out=dst[:, ssl], in_=ps[:])

            def transp(j):
                """V'[j] tiles from vT via PE transpose (both heads)."""
                tp = aux_ps.tile([128, 128], BF16, tag="aux")
                nc.tensor.transpose(tp[:], vT[:, j * JTL:(j + 1) * JTL], ident[:])
                nc.vector.tensor_copy(out=v0p[:, j, 0:DIM_HEAD], in_=tp[:, 0:DIM_HEAD])
                nc.vector.tensor_copy(out=v1p[:, j, 0:DIM_HEAD], in_=tp[:, DIM_HEAD:D2])

            states = {}

            def emit_epilogue_step(i_prev, step):
                """Deferred epilogue for slice i_prev (runs during slice
                i_prev+1): PSUM release, normalize, Wo + store."""
                e = states[i_prev]
                if step == 0:
                    e["av_sb0"] = n_sb.tile([DIM_HEAD + 1, ISL], F32, tag="av_sb0", name="av_sb0")
                    e["av_sb1"] = n_sb.tile([DIM_HEAD + 1, ISL], F32, tag="av_sb1", name="av_sb1")
                    nc.vector.tensor_copy(out=e["av_sb0"][:], in_=e["av0"][:])
                    nc.vector.tensor_copy(out=e["av_sb1"][:], in_=e["av1"][:])
                elif step == 1:
                    e["r0"] = n_sb.tile([1, ISL], F32, tag="r0", name="r0")
                    e["r1"] = n_sb.tile([1, ISL], F32, tag="r1", name="r1")
                    if USE_FAST_RECIP:
                        nc.vector.reciprocal_approx_fast(
                            out=e["r0"][:], in_=e["av_sb0"][DIM_HEAD:DIM_HEAD + 1, :])
                        nc.vector.reciprocal_approx_fast(
                            out=e["r1"][:], in_=e["av_sb1"][DIM_HEAD:DIM_HEAD + 1, :])
                    else:
                        nc.vector.reciprocal(e["r0"][:], e["av_sb0"][DIM_HEAD:DIM_HEAD + 1, :])
                        nc.vector.reciprocal(e["r1"][:], e["av_sb1"][DIM_HEAD:DIM_HEAD + 1, :])
                elif step == 2:
                    e["r0b"] = n_sb.tile([64, ISL], F32, tag="r0b", name="r0b")
                    e["r1b"] = n_sb.tile([64, ISL], F32, tag="r1b", name="r1b")
                    nc.gpsimd.partition_broadcast(e["r0b"][:], e["r0"][:])
                    nc.gpsimd.partition_broadcast(e["r1b"][:], e["r1"][:])
                elif step == 3:
                    e["lh"] = n_sb.tile([128, ISL], BF16, tag="lh", name="lh")
                    nc.vector.tensor_mul(out=e["lh"][0:64, :],
                                         in0=e["av_sb0"][0:DIM_HEAD, :], in1=e["r0b"][:])
                    nc.vector.tensor_mul(out=e["lh"][64:128, :],
                                         in0=e["av_sb1"][0:DIM_HEAD, :], in1=e["r1b"][:])
                else:  # steps 4..7: one Wo matmul + store each
                    s = step - 4
                    wop = aux_ps.tile([128, QD], F32, tag="aux")
                    nc.tensor.matmul(wop[:], e["lh"][:, s * 128:(s + 1) * 128],
                                     wo_sb[:], start=True, stop=True)
                    wos = n_sb.tile([128, QD], F32, tag="wos")
                    nc.vector.tensor_copy(out=wos[:], in_=wop[:])
                    nc.sync.dma_start(
                        out=out[i_prev * ISL + s * 128:i_prev * ISL + (s + 1) * 128, :],
                        in_=wos[:])

            # warm up slice 0 of each projection before the attention loop
            proj(wk_sb, kT, 0)
            proj(wq_sb, qT, 0)
            proj(wv_sb, vT, 0)
            for j in range(4):
                transp(j)

            # epilogue emission slots within the following slice's j-loop
            EPI = {0: 0, 2: 1, 4: 2, 6: 3, 16: 4, 18: 5, 20: 6, 22: 7}

            # Virtual-clock pacing for the Tile scheduler: floor each
            # iteration's readiness at its real-time slot so the scheduler
            # cannot hoist future slices' work (e.g. q-projection copies)
            # into earlier engine-queue positions where it head-blocks the
            # DVE FIFO and stalls the slice epilogue chain.
            HEAD_US = 14.0
            PACE0_US = 1.7   # slice 0 carries the k/v/V' prologue
            PACE_US = 1.25   # steady-state, exp-bound

            def slot_ms(i, j):
                t = HEAD_US + min(i, 1) * NJ * PACE0_US + max(i - 1, 0) * NJ * PACE_US
                return (t + min(j, NJ) * (PACE0_US if i == 0 else PACE_US)) / 1e3

            for i in range(NI):
                isl = slice(i * ISL, (i + 1) * ISL)
                av0 = av_ps.tile([DIM_HEAD + 1, ISL], F32, tag="av0")
                av1 = av_ps.tile([DIM_HEAD + 1, ISL], F32, tag="av1")
                pts = {}
                for j in range(NJ + LAG):
                    tc.tile_set_cur_wait(ms=slot_ms(i, j))
                    if j < NJ:
                        jsl = slice(j * JTL, (j + 1) * JTL)
                        st = st_ps.tile([128, 2 * ISL], F32, tag="st")
                        nc.tensor.matmul(st[:, 0:ISL], kT[0:64, jsl], qT[0:64, isl],
                                         start=True, stop=True)
                        nc.tensor.matmul(st[:, ISL:2 * ISL], kT[64:128, jsl],
                                         qT[64:128, isl], start=True, stop=True)
                        pt = p_sb.tile([128, 2 * ISL], BF16, tag="pt")
                        nc.scalar.activation(pt[:], st[:], EXP, scale=SCALE)
                        pts[j] = pt
                    if j >= LAG:
                        ja = j - LAG
                        pt = pts.pop(ja)
                        nc.tensor.matmul(av0[:], v0p[:, ja, :], pt[:, 0:ISL],
                                         start=(ja == 0), stop=(ja == NJ - 1))
                        nc.tensor.matmul(av1[:], v1p[:, ja, :], pt[:, ISL:2 * ISL],
                                         start=(ja == 0), stop=(ja == NJ - 1))
                    # prologue interleaves (i == 0): stream k/v/V' production
                    if i == 0 and j < NJ and j % 4 in (1, 2, 3):
                        s = j // 4 + 1
                        if s < NI:
                            if j % 4 == 1:
                                proj(wk_sb, kT, s)
                            elif j % 4 == 2:
                                proj(wv_sb, vT, s)
                            else:
                                for jj in range(4 * s, 4 * s + 4):
                                    transp(jj)
                    # deferred epilogue of slice i-1 (1-slice-deep pipeline)
                    if i > 0 and j in EPI:
                        emit_epilogue_step(i - 1, EPI[j])
                    # next slice's q projection (its DVE copy lands mid-slice,
                    # well before the next slice's first QK needs it)
                    if j == 8 and i + 1 < NI:
                        proj(wq_sb, qT, i + 1)
                states[i] = {"av0": av0, "av1": av1}

            # drain: full epilogue for the last slice
            tc.tile_set_cur_wait(ms=slot_ms(NI - 1, NJ + LAG))
            for step in range(8):
                emit_epilogue_step(NI - 1, step)

    nc.compile()
    return nc


_NC = None


def _get_program():
    global _NC
    if _NC is None:
        _NC = build_program()
    return _NC


def kernel(x, Wq, Wk, Wv, Wo, bo):
    x = np.asarray(x, dtype=np.float32)
    Wq = np.asarray(Wq, dtype=np.float32)
    Wk = np.asarray(Wk, dtype=np.float32)
    Wv = np.asarray(Wv, dtype=np.float32)
    Wo = np.asarray(Wo, dtype=np.float32)
    bo = np.asarray(bo, dtype=np.float32)

    nc = _get_program()

    in_maps = []
    for c in range(NCORES):
        b, m = divmod(c, NCORES // B)
        cs = slice(m * D2, (m + 1) * D2)
        in_maps.append({
            "xT": np.ascontiguousarray(x[b].T).astype(BFNP),
            "wq": np.ascontiguousarray(Wq[:, cs]).astype(BFNP),
            "wk": np.ascontiguousarray(Wk[:, cs]).astype(BFNP),
            "wv": np.ascontiguousarray(Wv[:, cs]).astype(BFNP),
            "wo": np.ascontiguousarray(Wo[cs, :]).astype(BFNP),
        })

    res = run_bass_kernel_spmd(nc, in_maps, core_ids=list(range(NCORES)))

    out = np.zeros((B, N, QD), dtype=np.float32)
    for c in range(NCORES):
        b = c // (NCORES // B)
        out[b] += res.results[c]["out"]
    out += bo[None, None, :]
    return out
